# revision 34
# baseline (speedup 1.0000x reference)
"""MultiHeadSelfAttentionWithRoPE on 8 TRN2 NeuronCores.

Sharding: every core computes full K/V (replicated); queries are sharded
round-robin (core c owns global rows {c, c+8, c+16, ...}) so causal
attention work is perfectly balanced and the program is identical across
cores (pure SPMD — per-core differences enter only through input data:
the pre-gathered query slice of x, per-core RoPE tables for those rows,
and a [128,16] causal boundary mask).

Layouts (transposed so no on-chip transposes are needed):
  xT   [768, 4096] bf16 (host-transposed x)
  QT/KT [128, eo(2), quad(3), n] bf16 — head-dim permutation (evens/odds
        split, 4 heads per 32-row group) folded into wq/wk column order
        on host; scores are invariant to a consistent q/k permutation.
  ST   = KT.T @ QT per 128-wide kv-block into PSUM (kv on partitions, q
        on free) as 2 accumulating K=32 matmuls (evens+odds) x 4 heads
        in distinct 32-row PE groups (concurrent row tiling).
  V    [4096, 768(+ones col per head)] bf16 normal orientation; the ones
        column makes the softmax denominator fall out of the PV matmul.
  OT   [65, 512] per head accumulated in PSUM over kv-blocks; row 64 is
        the denominator; normalize = reciprocal + partition_broadcast +
        multiply while copying rows 0..63 to SBUF bf16.
  out  = OT.T @ woT accumulated over 6 head-dim slabs, fp32.

Softmax skips max-subtraction (valid scores here are bounded |s| < ~8;
exp reads fp32 scores from PSUM). The q-slice of the ST/PV matmuls for
kv-block k is [16k, 512) — with stride-8 query interleave, q-col j only
attends kv-blocks k <= j//16, and only the first 16 cols of each block
are causally partial (masked with a per-core [128,16] 0/1 mask).
"""

import numpy as np

D = 768
S = 4096
H = 12
HD = 64
HD2 = 32
NCORES = 8
QPC = S // NCORES          # 512 query rows per core
NKV = S // 128             # 32 kv blocks
NCH = S // 512             # 8 seq chunks
NDS = D // 128             # 6 d-slices
SCALE = float(1.0 / np.sqrt(HD))

_CACHE = {}
last_exec_time_ns = None
last_results = None


def _head_perm():
    """Column permutation for wq/wk: slab s = 3*eo + quad, partition p =
    32*a + i  ->  original dim e = 64*(4*quad + a) + 2*i + eo."""
    perm = np.zeros(D, dtype=np.int64)
    for s in range(6):
        eo, quad = divmod(s, 3)
        for p in range(128):
            a, i = divmod(p, 32)
            perm[128 * s + p] = 64 * (4 * quad + a) + 2 * i + eo
    return perm


def _build_program(debug_taps=False):
    import concourse.mybir as mybir
    import concourse.tile as tile
    from concourse import bacc
    from contextlib import ExitStack

    dt = mybir.dt
    bf = dt.bfloat16
    f32 = dt.float32
    nc = bacc.Bacc("TRN2", target_bir_lowering=False, debug=False,
                   num_devices=NCORES)

    def din(name, shape, dtype):
        return nc.dram_tensor(name, shape, dtype, kind="ExternalInput").ap()

    xT_d = din("xT", [D, S], bf)
    xq_d = din("xq", [D, QPC], bf)
    wqT_d = din("wqT", [D, D], bf)     # permuted cols
    wkT_d = din("wkT", [D, D], bf)     # permuted cols
    wvT_d = din("wvT", [D, D], bf)     # natural
    woT_d = din("woT", [D, D], bf)     # wo.T natural
    cosK_d = din("cosK", [128, S], bf)
    sinK_d = din("sinK", [128, S], bf)
    cosQ_d = din("cosQ", [128, QPC], bf)
    sinQ_d = din("sinQ", [128, QPC], bf)
    mask_d = din("mask", [128, 16], bf)
    out_d = nc.dram_tensor("out", [QPC, D], f32, kind="ExternalOutput").ap()

    with tile.TileContext(nc) as tc, ExitStack() as ctx:
        # ---- long-lived SBUF (14.3 MB) ----
        P_LL = ctx.enter_context(tc.tile_pool(name="ll", bufs=1))
        KT = P_LL.tile([128, 2, 3, S], bf)                  # 6.3 MB
        QT = P_LL.tile([128, 2, 3, QPC], bf)                # 0.8 MB
        VSB = P_LL.tile([128, NKV, H, HD + 1], bf)          # 6.4 MB
        OTSB = P_LL.tile([128, 6, QPC], bf)                 # 0.8 MB
        msk = P_LL.tile([128, 16], bf)
        nc.sync.dma_start(msk[:], mask_d)
        nc.gpsimd.memset(VSB[:, :, :, HD:HD + 1], 1.0)

        # ---- transient SBUF pools ----
        P_W = ctx.enter_context(tc.tile_pool(name="wt", bufs=2))   # 2x1.2MB
        P_X = ctx.enter_context(tc.tile_pool(name="xs", bufs=12))
        P_RT = ctx.enter_context(tc.tile_pool(name="rt", bufs=3))
        P_CS = ctx.enter_context(tc.tile_pool(name="cs", bufs=3))
        P_P = ctx.enter_context(tc.tile_pool(name="pp", bufs=3))
        P_N = ctx.enter_context(tc.tile_pool(name="nrm", bufs=2))
        P_O = ctx.enter_context(tc.tile_pool(name="outs", bufs=2))

        def load_w(dram):
            # scalar-engine DMA queue: runs parallel to the sync-engine
            # queue that streams x chunks
            w = P_W.tile([128, NDS, D], bf, tag="w")
            for ds in range(NDS):
                nc.scalar.dma_start(w[:, ds, :],
                                    dram[128 * ds:128 * (ds + 1), :])
            return w

        def rope2(de, do, src_e_ps, src_o_ps, cos_t, sin_t, n):
            """de/do bf16 [128,n] <- rotate psum pair by (cos,sin) bf16.
            Stages psum->bf16 SBUF first so all tensor_tensor ops run in
            the DVE 16-bit fast path."""
            se = P_RT.tile([128, n], bf, tag="se")
            so = P_RT.tile([128, n], bf, tag="so")
            nc.vector.tensor_copy(se[:], src_e_ps[:])
            nc.vector.tensor_copy(so[:], src_o_ps[:])
            t1 = P_RT.tile([128, n], bf, tag="t1")
            t2 = P_RT.tile([128, n], bf, tag="t2")
            nc.vector.tensor_mul(t1[:], se[:], cos_t[:])
            nc.vector.tensor_mul(t2[:], so[:], sin_t[:])
            nc.vector.tensor_sub(de, t1[:], t2[:])
            t3 = P_RT.tile([128, n], bf, tag="t1")
            t4 = P_RT.tile([128, n], bf, tag="t2")
            nc.vector.tensor_mul(t3[:], se[:], sin_t[:])
            nc.vector.tensor_mul(t4[:], so[:], cos_t[:])
            nc.vector.tensor_add(do, t3[:], t4[:])

        # ============ phases A+B: projections (psum: 2+2+2 banks) ======
        with tc.tile_pool(name="pps", bufs=2, space="PSUM") as P_PS:

            def load_chunk(ch):
                c0 = 512 * ch
                xts = []
                for ds in range(NDS):
                    xt = P_X.tile([128, 512], bf, tag="xt",
                                  name=f"xt{ch}_{ds}")
                    nc.sync.dma_start(
                        xt[:], xT_d[128 * ds:128 * (ds + 1), c0:c0 + 512])
                    xts.append(xt)
                ck = P_CS.tile([128, 512], bf, tag="ck", name=f"ck{ch}")
                sk = P_CS.tile([128, 512], bf, tag="sk", name=f"sk{ch}")
                nc.sync.dma_start(ck[:], cosK_d[:, c0:c0 + 512])
                nc.sync.dma_start(sk[:], sinK_d[:, c0:c0 + 512])
                return xts, ck, sk

            nxt = load_chunk(0)
            wk_sb = load_w(wkT_d)
            wv_sb = load_w(wvT_d)

            def qk_proj(w_sb, xtiles, dst, c0, n, cos_t, sin_t):
                """Project + rope one 512-col chunk into dst KT/QT."""
                for quad in range(3):
                    pe = P_PS.tile([128, n], f32, tag="kpsE")
                    po = P_PS.tile([128, n], f32, tag="kpsO")
                    for s, ps in ((quad, pe), (3 + quad, po)):
                        for ds in range(NDS):
                            nc.tensor.matmul(
                                ps[:], w_sb[:, ds, 128 * s:128 * (s + 1)],
                                xtiles[ds][:], start=(ds == 0),
                                stop=(ds == NDS - 1))
                    rope2(dst[:, 0, quad, c0:c0 + n], dst[:, 1, quad, c0:c0 + n],
                          pe, po, cos_t, sin_t, n)

            for ch in range(NCH):
                c0 = 512 * ch
                xts, ck, sk = nxt
                if ch + 1 < NCH:
                    nxt = load_chunk(ch + 1)

                qk_proj(wk_sb, xts, KT, c0, 512, ck, sk)

                # V projection (normal orientation), 4 seq sub-blocks
                for sb in range(4):
                    k_blk = 4 * ch + sb
                    for nh in range(2):
                        vps = P_PS.tile([128, 384], f32, tag=f"vps{nh}")
                        for ds in range(NDS):
                            nc.tensor.matmul(
                                vps[:], xts[ds][:, 128 * sb:128 * (sb + 1)],
                                wv_sb[:, ds, 384 * nh:384 * (nh + 1)],
                                start=(ds == 0), stop=(ds == NDS - 1))
                        nc.scalar.copy(
                            VSB[:, k_blk, 6 * nh:6 * (nh + 1), 0:HD],
                            vps[:].rearrange("p (h d) -> p h d", h=6))

            # ---- Q projection ----
            wq_sb = load_w(wqT_d)
            cq = P_CS.tile([128, QPC], bf, tag="ck")
            sq = P_CS.tile([128, QPC], bf, tag="sk")
            nc.sync.dma_start(cq[:], cosQ_d)
            nc.sync.dma_start(sq[:], sinQ_d)
            xqs = []
            for ds in range(NDS):
                xq = P_X.tile([128, QPC], bf, tag="xt")
                nc.sync.dma_start(xq[:], xq_d[128 * ds:128 * (ds + 1), :])
                xqs.append(xq)
            qk_proj(wq_sb, xqs, QT, 0, QPC, cq, sq)

        # Load wo early so its DMA overlaps attention.
        wo_sb = load_w(woT_d)
        nrm_d = nc.dram_tensor("nrm_scratch", [3, 4 * QPC], f32,
                               kind="Internal").ap()

        # ============ phase C: attention (psum: 2+2 ST + 4 OT banks) ===
        # ST is split into two 2-bank half-tiles (heads {0,1} / {2,3});
        # exp of one half overlaps the ST matmuls of the other, so the
        # PE never waits for the ACT engine (the single-tile version
        # serialized ST(k+1) behind exp(k) on the shared banks). PV for
        # kv-block k-1 is emitted after the ST matmuls of block k.
        with tc.tile_pool(name="st", bufs=1, space="PSUM") as P_ST, \
             tc.tile_pool(name="ot", bufs=1, space="PSUM") as P_OT:
            for g in range(3):                  # head quads
                otb = P_OT.tile([65, 4, QPC], f32, tag="ot")
                prev = None

                def pv_flush(g=g, otb=otb):
                    nonlocal prev
                    if prev is None:
                        return
                    pk, pps = prev
                    pq0 = 16 * pk
                    for a in range(4):
                        nc.tensor.matmul(
                            otb[:, a, pq0:QPC], VSB[:, pk, 4 * g + a, :],
                            pps[a // 2][:, a % 2, :], start=(pk == 0),
                            stop=(pk == NKV - 1))
                    prev = None

                for k in range(NKV):            # kv blocks
                    q0 = 16 * k
                    n = QPC - q0
                    halves = []
                    for hb in range(2):         # half: heads {2hb, 2hb+1}
                        stb = P_ST.tile([128, 2, 512], f32, tag=f"st{hb}")
                        for eo in range(2):
                            for aa in range(2):
                                a = 2 * hb + aa
                                tp = (96, 0) if a == 3 else None
                                nc.tensor.matmul(
                                    stb[:, aa, 0:n],
                                    KT[32 * a:32 * (a + 1), eo, g,
                                       128 * k:128 * (k + 1)],
                                    QT[32 * a:32 * (a + 1), eo, g, q0:QPC],
                                    start=(eo == 0), stop=(eo == 1),
                                    tile_position=tp)
                        halves.append(stb)
                    pv_flush()
                    pps = []
                    for hb in range(2):
                        p = P_P.tile([128, 2, n], bf, tag=f"p{hb}")
                        nc.scalar.activation(
                            p[:], halves[hb][:, :, 0:n],
                            mybir.ActivationFunctionType.Exp, scale=SCALE)
                        nc.vector.tensor_mul(
                            p[:, :, 0:16], p[:, :, 0:16],
                            msk[:, None, :].broadcast_to((128, 2, 16)))
                        pps.append(p)
                    prev = (k, pps)
                pv_flush()

                # Free the OT banks fast, all on DVE (ACT stays free for
                # the next quad's exps): reciprocals + unnormalized copies.
                # The broadcast DMA bounce + in-place normalize overlap
                # the next quad's attention.
                r1 = P_N.tile([1, 4 * QPC], f32, tag="r1")
                for a in range(4):
                    nc.vector.reciprocal(r1[:, QPC * a:QPC * (a + 1)],
                                         otb[64:65, a, :])
                for half in range(2):
                    nc.vector.tensor_copy(
                        OTSB[64 * half:64 * half + 64, 2 * g:2 * g + 2, :],
                        otb[0:64, half::2, :])
                nc.sync.dma_start(nrm_d[g:g + 1, :], r1[:])
                rb = P_N.tile([128, 4 * QPC], f32, tag="rb")
                nc.sync.dma_start(rb[:],
                                  nrm_d[g:g + 1, :].to_broadcast((128, 4 * QPC)))
                rb4 = rb[:].rearrange("p (a q) -> p a q", a=4)
                for a in range(4):
                    h = 4 * g + a
                    pb = 64 * (h % 2)
                    dst = OTSB[pb:pb + 64, h // 2, :]
                    nc.vector.tensor_mul(dst, dst, rb4[pb:pb + 64, a, :])

        if debug_taps:
            kt_t = nc.dram_tensor("dbg_kt", [128, 2 * 3 * S], dt.bfloat16,
                                  kind="ExternalOutput").ap()
            qt_t = nc.dram_tensor("dbg_qt", [128, 2 * 3 * QPC], dt.bfloat16,
                                  kind="ExternalOutput").ap()
            v_t = nc.dram_tensor("dbg_v", [128, NKV * H * (HD + 1)],
                                 dt.bfloat16, kind="ExternalOutput").ap()
            ot_t = nc.dram_tensor("dbg_ot", [128, 6 * QPC], dt.bfloat16,
                                  kind="ExternalOutput").ap()
            nc.sync.dma_start(kt_t, KT[:].rearrange("p a b c -> p (a b c)"))
            nc.sync.dma_start(qt_t, QT[:].rearrange("p a b c -> p (a b c)"))
            nc.sync.dma_start(v_t, VSB[:].rearrange("p a b c -> p (a b c)"))
            nc.sync.dma_start(ot_t, OTSB[:].rearrange("p a b -> p (a b)"))

        # ============ phase D: output projection =======================
        with tc.tile_pool(name="pd", bufs=1, space="PSUM") as P_PD:
            for j in range(4):                  # q sub-tiles of 128
                pss = []
                for nh in range(2):
                    ps = P_PD.tile([128, 384], f32, tag=f"ops{nh}")
                    for s in range(NDS):
                        nc.tensor.matmul(
                            ps[:], OTSB[:, s, 128 * j:128 * (j + 1)],
                            wo_sb[:, s, 384 * nh:384 * (nh + 1)],
                            start=(s == 0), stop=(s == NDS - 1))
                    pss.append(ps)
                ob = P_O.tile([128, D], f32, tag="ob")
                nc.scalar.copy(ob[:, 0:384], pss[0][:])
                nc.scalar.copy(ob[:, 384:768], pss[1][:])
                nc.sync.dma_start(out_d[128 * j:128 * (j + 1), :], ob[:])

    nc.compile()
    return nc


def _prep_inputs(x, wq, wk, wv, wo, token_positions):
    import ml_dtypes
    bf16 = ml_dtypes.bfloat16

    x2 = np.ascontiguousarray(x[0], dtype=np.float32)          # [S, D]
    xT = np.ascontiguousarray(x2.T).astype(bf16)               # [D, S]
    perm = _head_perm()
    wqT = np.ascontiguousarray(wq[perm, :].T).astype(bf16)     # [d, perm e]
    wkT = np.ascontiguousarray(wk[perm, :].T).astype(bf16)
    wvT = np.ascontiguousarray(wv.T).astype(bf16)
    woT = np.ascontiguousarray(wo.T).astype(bf16)

    pos = np.asarray(token_positions[0], dtype=np.int64)       # [S]
    kk = np.arange(HD2, dtype=np.float32)
    inv = (10000.0 ** (-2.0 * kk / HD)).astype(np.float32)
    ang = pos[:, None].astype(np.float32) * inv[None, :]       # [S, 32]
    cosf = np.cos(ang, dtype=np.float32)
    sinf = np.sin(ang, dtype=np.float32)
    cosK = np.ascontiguousarray(np.tile(cosf.T, (4, 1))).astype(bf16)
    sinK = np.ascontiguousarray(np.tile(sinf.T, (4, 1))).astype(bf16)

    per_core = []
    for c in range(NCORES):
        xq = np.ascontiguousarray(xT[:, c::NCORES])            # [D, 512]
        cosQ = np.ascontiguousarray(
            np.tile(cosf[c::NCORES].T, (4, 1))).astype(bf16)
        sinQ = np.ascontiguousarray(
            np.tile(sinf[c::NCORES].T, (4, 1))).astype(bf16)
        kl = np.arange(128)[:, None]
        jj = np.arange(16)[None, :]
        mask = (kl <= 8 * jj + c).astype(np.float32).astype(bf16)
        per_core.append({
            "xT": xT, "xq": xq,
            "wqT": wqT, "wkT": wkT, "wvT": wvT, "woT": woT,
            "cosK": cosK, "sinK": sinK, "cosQ": cosQ, "sinQ": sinQ,
            "mask": mask,
        })
    return per_core


def kernel(x, wq, wk, wv, wo, token_positions):
    global last_exec_time_ns, last_results
    import os
    from concourse import bass_utils

    key = "v1"
    if key not in _CACHE:
        _CACHE[key] = _build_program()
    nc = _CACHE[key]

    in_maps = _prep_inputs(np.asarray(x), np.asarray(wq), np.asarray(wk),
                           np.asarray(wv), np.asarray(wo),
                           np.asarray(token_positions))

    kw = {}
    if os.environ.get("BASS_KERNEL_TRACE", "0") == "1":
        kw = dict(trace=True,
                  trace_cores=[int(t) for t in os.environ.get(
                      "BASS_KERNEL_TRACE_CORES", "0").split(",")])
    res = bass_utils.run_bass_kernel_spmd(nc, in_maps,
                                          core_ids=list(range(NCORES)), **kw)
    last_exec_time_ns = res.exec_time_ns
    last_results = res

    out = np.empty((S, D), dtype=np.float32)
    for c in range(NCORES):
        out[c::NCORES, :] = res.results[c]["out"]
    return out[None, :, :]


# revision 35
# speedup vs baseline: 1.1869x; 1.1869x over previous
"""MultiHeadSelfAttentionWithRoPE on 8 TRN2 NeuronCores.

Sharding: every core computes full K/V (replicated); queries are sharded
round-robin (core c owns global rows {c, c+8, c+16, ...}) so causal
attention work is perfectly balanced and the program is identical across
cores (pure SPMD — per-core differences enter only through input data:
the pre-gathered query slice of x, per-core RoPE tables for those rows,
and a [128,16] causal boundary mask).

Layouts (transposed so no on-chip transposes are needed):
  xT   [768, 4096] bf16 (host-transposed x)
  QT/KT [128, eo(2), quad(3), n] bf16 — head-dim permutation (evens/odds
        split, 4 heads per 32-row group) folded into wq/wk column order
        on host; scores are invariant to a consistent q/k permutation.
  ST   = KT.T @ QT per 128-wide kv-block into PSUM (kv on partitions, q
        on free) as 2 accumulating K=32 matmuls (evens+odds) x 4 heads
        in distinct 32-row PE groups (concurrent row tiling).
  V    [4096, 768(+ones col per head)] bf16 normal orientation; the ones
        column makes the softmax denominator fall out of the PV matmul.
  OT   [65, 512] per head accumulated in PSUM over kv-blocks; row 64 is
        the denominator; normalize = reciprocal + partition_broadcast +
        multiply while copying rows 0..63 to SBUF bf16.
  out  = OT.T @ woT accumulated over 6 head-dim slabs, fp32.

Softmax skips max-subtraction (valid scores here are bounded |s| < ~8;
exp reads fp32 scores from PSUM). The q-slice of the ST/PV matmuls for
kv-block k is [16k, 512) — with stride-8 query interleave, q-col j only
attends kv-blocks k <= j//16, and only the first 16 cols of each block
are causally partial (masked with a per-core [128,16] 0/1 mask).
"""

import numpy as np

D = 768
S = 4096
H = 12
HD = 64
HD2 = 32
NCORES = 8
QPC = S // NCORES          # 512 query rows per core
NKV = S // 128             # 32 kv blocks
NCH = S // 512             # 8 seq chunks
NDS = D // 128             # 6 d-slices
SCALE = float(1.0 / np.sqrt(HD))

_CACHE = {}
last_exec_time_ns = None
last_results = None


def _head_perm():
    """Column permutation for wq/wk: slab s = 3*eo + quad, partition p =
    32*a + i  ->  original dim e = 64*(4*quad + a) + 2*i + eo."""
    perm = np.zeros(D, dtype=np.int64)
    for s in range(6):
        eo, quad = divmod(s, 3)
        for p in range(128):
            a, i = divmod(p, 32)
            perm[128 * s + p] = 64 * (4 * quad + a) + 2 * i + eo
    return perm


def _build_program(debug_taps=False):
    import concourse.mybir as mybir
    import concourse.tile as tile
    from concourse import bacc
    from contextlib import ExitStack

    dt = mybir.dt
    bf = dt.bfloat16
    f32 = dt.float32
    nc = bacc.Bacc("TRN2", target_bir_lowering=False, debug=False,
                   num_devices=NCORES)

    def din(name, shape, dtype):
        return nc.dram_tensor(name, shape, dtype, kind="ExternalInput").ap()

    xT_d = din("xT", [D, S], bf)
    xq_d = din("xq", [D, QPC], bf)
    wqT_d = din("wqT", [D, D], bf)     # permuted cols
    wkT_d = din("wkT", [D, D], bf)     # permuted cols
    wvT_d = din("wvT", [D, D], bf)     # natural
    woT_d = din("woT", [D, D], bf)     # wo.T natural
    cosK_d = din("cosK", [128, S], bf)
    sinK_d = din("sinK", [128, S], bf)
    cosQ_d = din("cosQ", [128, QPC], bf)
    sinQ_d = din("sinQ", [128, QPC], bf)
    mask_d = din("mask", [128, 16], bf)
    out_d = nc.dram_tensor("out", [QPC, D], f32, kind="ExternalOutput").ap()

    with tile.TileContext(nc) as tc, ExitStack() as ctx:
        # ---- long-lived SBUF (14.3 MB) ----
        P_LL = ctx.enter_context(tc.tile_pool(name="ll", bufs=1))
        KT = P_LL.tile([128, 2, 3, S], bf)                  # 6.3 MB
        QT = P_LL.tile([128, 2, 3, QPC], bf)                # 0.8 MB
        VSB = P_LL.tile([128, NKV, H, HD + 1], bf)          # 6.4 MB
        OTSB = P_LL.tile([128, 6, QPC], bf)                 # 0.8 MB
        msk = P_LL.tile([128, 16], bf)
        nc.sync.dma_start(msk[:], mask_d)
        nc.gpsimd.memset(VSB[:, :, :, HD:HD + 1], 1.0)

        # ---- transient SBUF pools ----
        P_W = ctx.enter_context(tc.tile_pool(name="wt", bufs=2))   # 2x1.2MB
        P_X = ctx.enter_context(tc.tile_pool(name="xs", bufs=12))
        P_RT = ctx.enter_context(tc.tile_pool(name="rt", bufs=3))
        P_CS = ctx.enter_context(tc.tile_pool(name="cs", bufs=3))
        P_P = ctx.enter_context(tc.tile_pool(name="pp", bufs=3))
        P_N = ctx.enter_context(tc.tile_pool(name="nrm", bufs=2))
        P_O = ctx.enter_context(tc.tile_pool(name="outs", bufs=2))

        def load_w(dram):
            # scalar-engine DMA queue: runs parallel to the sync-engine
            # queue that streams x chunks
            w = P_W.tile([128, NDS, D], bf, tag="w")
            for ds in range(NDS):
                nc.scalar.dma_start(w[:, ds, :],
                                    dram[128 * ds:128 * (ds + 1), :])
            return w

        def rope2(de, do, src_e_ps, src_o_ps, cos_t, sin_t, n):
            """de/do bf16 [128,n] <- rotate psum pair by (cos,sin) bf16.
            Stages psum->bf16 SBUF first so all tensor_tensor ops run in
            the DVE 16-bit fast path."""
            se = P_RT.tile([128, n], bf, tag="se")
            so = P_RT.tile([128, n], bf, tag="so")
            nc.vector.tensor_copy(se[:], src_e_ps[:])
            nc.vector.tensor_copy(so[:], src_o_ps[:])
            t1 = P_RT.tile([128, n], bf, tag="t1")
            t2 = P_RT.tile([128, n], bf, tag="t2")
            nc.vector.tensor_mul(t1[:], se[:], cos_t[:])
            nc.vector.tensor_mul(t2[:], so[:], sin_t[:])
            nc.vector.tensor_sub(de, t1[:], t2[:])
            t3 = P_RT.tile([128, n], bf, tag="t1")
            t4 = P_RT.tile([128, n], bf, tag="t2")
            nc.vector.tensor_mul(t3[:], se[:], sin_t[:])
            nc.vector.tensor_mul(t4[:], so[:], cos_t[:])
            nc.vector.tensor_add(do, t3[:], t4[:])

        # ============ phases A+B: projections (psum: 2+2+2 banks) ======
        with tc.tile_pool(name="pps", bufs=2, space="PSUM") as P_PS:

            def load_chunk(ch):
                c0 = 512 * ch
                xts = []
                for ds in range(NDS):
                    xt = P_X.tile([128, 512], bf, tag="xt",
                                  name=f"xt{ch}_{ds}")
                    nc.sync.dma_start(
                        xt[:], xT_d[128 * ds:128 * (ds + 1), c0:c0 + 512])
                    xts.append(xt)
                ck = P_CS.tile([128, 512], bf, tag="ck", name=f"ck{ch}")
                sk = P_CS.tile([128, 512], bf, tag="sk", name=f"sk{ch}")
                nc.sync.dma_start(ck[:], cosK_d[:, c0:c0 + 512])
                nc.sync.dma_start(sk[:], sinK_d[:, c0:c0 + 512])
                return xts, ck, sk

            nxt = load_chunk(0)
            wk_sb = load_w(wkT_d)
            wv_sb = load_w(wvT_d)

            def qk_proj(w_sb, xtiles, dst, c0, n, cos_t, sin_t):
                """Project + rope one 512-col chunk into dst KT/QT."""
                for quad in range(3):
                    pe = P_PS.tile([128, n], f32, tag="kpsE")
                    po = P_PS.tile([128, n], f32, tag="kpsO")
                    for s, ps in ((quad, pe), (3 + quad, po)):
                        for ds in range(NDS):
                            nc.tensor.matmul(
                                ps[:], w_sb[:, ds, 128 * s:128 * (s + 1)],
                                xtiles[ds][:], start=(ds == 0),
                                stop=(ds == NDS - 1))
                    rope2(dst[:, 0, quad, c0:c0 + n], dst[:, 1, quad, c0:c0 + n],
                          pe, po, cos_t, sin_t, n)

            for ch in range(NCH):
                c0 = 512 * ch
                xts, ck, sk = nxt
                if ch + 1 < NCH:
                    nxt = load_chunk(ch + 1)

                qk_proj(wk_sb, xts, KT, c0, 512, ck, sk)

                # V projection (normal orientation), 4 seq sub-blocks
                for sb in range(4):
                    k_blk = 4 * ch + sb
                    for nh in range(2):
                        vps = P_PS.tile([128, 384], f32, tag="vps")
                        for ds in range(NDS):
                            nc.tensor.matmul(
                                vps[:], xts[ds][:, 128 * sb:128 * (sb + 1)],
                                wv_sb[:, ds, 384 * nh:384 * (nh + 1)],
                                start=(ds == 0), stop=(ds == NDS - 1))
                        nc.scalar.copy(
                            VSB[:, k_blk, 6 * nh:6 * (nh + 1), 0:HD],
                            vps[:].rearrange("p (h d) -> p h d", h=6))

            # ---- Q projection ----
            wq_sb = load_w(wqT_d)
            cq = P_CS.tile([128, QPC], bf, tag="ck")
            sq = P_CS.tile([128, QPC], bf, tag="sk")
            nc.sync.dma_start(cq[:], cosQ_d)
            nc.sync.dma_start(sq[:], sinQ_d)
            xqs = []
            for ds in range(NDS):
                xq = P_X.tile([128, QPC], bf, tag="xt")
                nc.sync.dma_start(xq[:], xq_d[128 * ds:128 * (ds + 1), :])
                xqs.append(xq)
            qk_proj(wq_sb, xqs, QT, 0, QPC, cq, sq)

        # Load wo early so its DMA overlaps attention.
        wo_sb = load_w(woT_d)
        nrm_d = nc.dram_tensor("nrm_scratch", [3, 4 * QPC], f32,
                               kind="Internal").ap()

        # ============ phase C: attention (psum: 2+2 ST + 4 OT banks) ===
        # ST is split into two 2-bank half-tiles (heads {0,1} / {2,3});
        # exp of one half overlaps the ST matmuls of the other, so the
        # PE never waits for the ACT engine (the single-tile version
        # serialized ST(k+1) behind exp(k) on the shared banks). PV for
        # kv-block k-1 is emitted after the ST matmuls of block k.
        with tc.tile_pool(name="st", bufs=1, space="PSUM") as P_ST, \
             tc.tile_pool(name="ot", bufs=1, space="PSUM") as P_OT:
            for g in range(3):                  # head quads
                otb = P_OT.tile([65, 4, QPC], f32, tag="ot")
                prev = None

                def pv_flush(g=g, otb=otb):
                    nonlocal prev
                    if prev is None:
                        return
                    pk, pps = prev
                    pq0 = 16 * pk
                    for a in range(4):
                        nc.tensor.matmul(
                            otb[:, a, pq0:QPC], VSB[:, pk, 4 * g + a, :],
                            pps[a // 2][:, a % 2, :], start=(pk == 0),
                            stop=(pk == NKV - 1))
                    prev = None

                for k in range(NKV):            # kv blocks
                    q0 = 16 * k
                    n = QPC - q0
                    halves = []
                    for hb in range(2):         # half: heads {2hb, 2hb+1}
                        stb = P_ST.tile([128, 2, 512], f32, tag=f"st{hb}")
                        for eo in range(2):
                            for aa in range(2):
                                a = 2 * hb + aa
                                tp = (96, 0) if a == 3 else None
                                nc.tensor.matmul(
                                    stb[:, aa, 0:n],
                                    KT[32 * a:32 * (a + 1), eo, g,
                                       128 * k:128 * (k + 1)],
                                    QT[32 * a:32 * (a + 1), eo, g, q0:QPC],
                                    start=(eo == 0), stop=(eo == 1),
                                    tile_position=tp)
                        halves.append(stb)
                    pv_flush()
                    pps = []
                    for hb in range(2):
                        p = P_P.tile([128, 2, n], bf, tag=f"p{hb}")
                        nc.scalar.activation(
                            p[:], halves[hb][:, :, 0:n],
                            mybir.ActivationFunctionType.Exp, scale=SCALE)
                        nc.vector.tensor_mul(
                            p[:, :, 0:16], p[:, :, 0:16],
                            msk[:, None, :].broadcast_to((128, 2, 16)))
                        pps.append(p)
                    prev = (k, pps)
                pv_flush()

                # Free the OT banks fast, all on DVE (ACT stays free for
                # the next quad's exps): reciprocals + unnormalized copies.
                # The broadcast DMA bounce + in-place normalize overlap
                # the next quad's attention.
                r1 = P_N.tile([1, 4 * QPC], f32, tag="r1")
                for a in range(4):
                    nc.vector.reciprocal(r1[:, QPC * a:QPC * (a + 1)],
                                         otb[64:65, a, :])
                for half in range(2):
                    nc.vector.tensor_copy(
                        OTSB[64 * half:64 * half + 64, 2 * g:2 * g + 2, :],
                        otb[0:64, half::2, :])
                nc.sync.dma_start(nrm_d[g:g + 1, :], r1[:])
                rb = P_N.tile([128, 4 * QPC], f32, tag="rb")
                nc.sync.dma_start(rb[:],
                                  nrm_d[g:g + 1, :].to_broadcast((128, 4 * QPC)))
                rb4 = rb[:].rearrange("p (a q) -> p a q", a=4)
                for a in range(4):
                    h = 4 * g + a
                    pb = 64 * (h % 2)
                    dst = OTSB[pb:pb + 64, h // 2, :]
                    nc.vector.tensor_mul(dst, dst, rb4[pb:pb + 64, a, :])

        if debug_taps:
            kt_t = nc.dram_tensor("dbg_kt", [128, 2 * 3 * S], dt.bfloat16,
                                  kind="ExternalOutput").ap()
            qt_t = nc.dram_tensor("dbg_qt", [128, 2 * 3 * QPC], dt.bfloat16,
                                  kind="ExternalOutput").ap()
            v_t = nc.dram_tensor("dbg_v", [128, NKV * H * (HD + 1)],
                                 dt.bfloat16, kind="ExternalOutput").ap()
            ot_t = nc.dram_tensor("dbg_ot", [128, 6 * QPC], dt.bfloat16,
                                  kind="ExternalOutput").ap()
            nc.sync.dma_start(kt_t, KT[:].rearrange("p a b c -> p (a b c)"))
            nc.sync.dma_start(qt_t, QT[:].rearrange("p a b c -> p (a b c)"))
            nc.sync.dma_start(v_t, VSB[:].rearrange("p a b c -> p (a b c)"))
            nc.sync.dma_start(ot_t, OTSB[:].rearrange("p a b -> p (a b)"))

        # ============ phase D: output projection =======================
        with tc.tile_pool(name="pd", bufs=1, space="PSUM") as P_PD:
            for j in range(4):                  # q sub-tiles of 128
                pss = []
                for nh in range(2):
                    ps = P_PD.tile([128, 384], f32, tag=f"ops{nh}")
                    for s in range(NDS):
                        nc.tensor.matmul(
                            ps[:], OTSB[:, s, 128 * j:128 * (j + 1)],
                            wo_sb[:, s, 384 * nh:384 * (nh + 1)],
                            start=(s == 0), stop=(s == NDS - 1))
                    pss.append(ps)
                ob = P_O.tile([128, D], f32, tag="ob")
                nc.scalar.copy(ob[:, 0:384], pss[0][:])
                nc.scalar.copy(ob[:, 384:768], pss[1][:])
                nc.sync.dma_start(out_d[128 * j:128 * (j + 1), :], ob[:])

    nc.compile()
    return nc


def _prep_inputs(x, wq, wk, wv, wo, token_positions):
    import ml_dtypes
    bf16 = ml_dtypes.bfloat16

    x2 = np.ascontiguousarray(x[0], dtype=np.float32)          # [S, D]
    xT = np.ascontiguousarray(x2.T).astype(bf16)               # [D, S]
    perm = _head_perm()
    wqT = np.ascontiguousarray(wq[perm, :].T).astype(bf16)     # [d, perm e]
    wkT = np.ascontiguousarray(wk[perm, :].T).astype(bf16)
    wvT = np.ascontiguousarray(wv.T).astype(bf16)
    woT = np.ascontiguousarray(wo.T).astype(bf16)

    pos = np.asarray(token_positions[0], dtype=np.int64)       # [S]
    kk = np.arange(HD2, dtype=np.float32)
    inv = (10000.0 ** (-2.0 * kk / HD)).astype(np.float32)
    ang = pos[:, None].astype(np.float32) * inv[None, :]       # [S, 32]
    cosf = np.cos(ang, dtype=np.float32)
    sinf = np.sin(ang, dtype=np.float32)
    cosK = np.ascontiguousarray(np.tile(cosf.T, (4, 1))).astype(bf16)
    sinK = np.ascontiguousarray(np.tile(sinf.T, (4, 1))).astype(bf16)

    per_core = []
    for c in range(NCORES):
        xq = np.ascontiguousarray(xT[:, c::NCORES])            # [D, 512]
        cosQ = np.ascontiguousarray(
            np.tile(cosf[c::NCORES].T, (4, 1))).astype(bf16)
        sinQ = np.ascontiguousarray(
            np.tile(sinf[c::NCORES].T, (4, 1))).astype(bf16)
        kl = np.arange(128)[:, None]
        jj = np.arange(16)[None, :]
        mask = (kl <= 8 * jj + c).astype(np.float32).astype(bf16)
        per_core.append({
            "xT": xT, "xq": xq,
            "wqT": wqT, "wkT": wkT, "wvT": wvT, "woT": woT,
            "cosK": cosK, "sinK": sinK, "cosQ": cosQ, "sinQ": sinQ,
            "mask": mask,
        })
    return per_core


def kernel(x, wq, wk, wv, wo, token_positions):
    global last_exec_time_ns, last_results
    import os
    from concourse import bass_utils

    key = "v1"
    if key not in _CACHE:
        _CACHE[key] = _build_program()
    nc = _CACHE[key]

    in_maps = _prep_inputs(np.asarray(x), np.asarray(wq), np.asarray(wk),
                           np.asarray(wv), np.asarray(wo),
                           np.asarray(token_positions))

    kw = {}
    if os.environ.get("BASS_KERNEL_TRACE", "0") == "1":
        kw = dict(trace=True,
                  trace_cores=[int(t) for t in os.environ.get(
                      "BASS_KERNEL_TRACE_CORES", "0").split(",")])
    res = bass_utils.run_bass_kernel_spmd(nc, in_maps,
                                          core_ids=list(range(NCORES)), **kw)
    last_exec_time_ns = res.exec_time_ns
    last_results = res

    out = np.empty((S, D), dtype=np.float32)
    for c in range(NCORES):
        out[c::NCORES, :] = res.results[c]["out"]
    return out[None, :, :]
